# revision 23
# baseline (speedup 1.0000x reference)
"""HNHN layer (hypergraph message passing) on 8 Trainium2 NeuronCores.

Math (reference, with B1 the binary node-edge incidence matrix):
    edge_card = colsum(B1)^-1.5          node_card = rowsum(B1)^-0.5
    d0_inv    = 1/(B1 @ edge_card)       d1_inv    = 1/(B1^T @ node_card)
    x1     = d1_inv * (B1^T @ (node_card * (x0 @ W0))) + b01
    x0_out = d0_inv * (B1  @ (edge_card * (x1 @ W1))) + b10
    returns (relu(x0_out), relu(x1))

Implementation: nodes are row-sharded 8 ways. Per core, three streaming
passes over the (bf16, exact for 0/1 values) incidence shard:
  P1: row sums (DVE reduce)  -> node_card
  P2: U = B1_shard^T @ [node_card*y | node_card | 1]   (PE, psum-accum)
      -> ReduceScatter(add) over cores gives each core its edge shard of
         [B1^T Yp | d1 | colsums]
  P3: V = B1_shard @ [edge_card*z | edge_card]          (PE, psum-accum)
      where z = x1_shard @ W1 is computed on the edge shard and
      AllGather'ed (bf16) to every core.
Stats ride along as extra matmul columns so no separate matvec passes are
needed. Only host-side prep: sharding, transposes, bf16 casts.
"""

import numpy as np
import ml_dtypes

import concourse.bass as bass
import concourse.mybir as mybir
import concourse.tile as tile
from concourse import bacc
from concourse.bass_utils import run_bass_kernel_spmd
from concourse.masks import make_identity

BF16 = ml_dtypes.bfloat16

N_NODES, N_EDGES = 16384, 8192
CH = 256
N_CORES = 8
NS = N_NODES // N_CORES      # 2048 nodes per core
ES = N_EDGES // N_CORES      # 1024 edges per core (edge shard)
P = 128
NB = NS // P                 # 16 node blocks / core
EBF = N_EDGES // P           # 64 edge blocks (full)
EBS = ES // P                # 8 edge blocks (shard)
F32 = mybir.dt.float32
BF = mybir.dt.bfloat16
AX = mybir.AxisListType
ALU = mybir.AluOpType
GROUPS = [list(range(N_CORES))]


def build_bass(reps=1, bench=False):
    nc = bacc.Bacc("TRN2", target_bir_lowering=False, debug=False,
                   num_devices=N_CORES)
    b1r = nc.dram_tensor("b1r", [NS, N_EDGES], BF, kind="ExternalInput")
    b1t = nc.dram_tensor("b1t", [N_EDGES, NS], BF, kind="ExternalInput")
    x0t = nc.dram_tensor("x0t", [CH, NS], F32, kind="ExternalInput")
    w0 = nc.dram_tensor("w0", [CH, CH], F32, kind="ExternalInput")
    w1 = nc.dram_tensor("w1", [CH, CH], F32, kind="ExternalInput")
    b01 = nc.dram_tensor("b01", [P, CH], F32, kind="ExternalInput")
    b10 = nc.dram_tensor("b10", [P, CH], F32, kind="ExternalInput")
    okind = "Internal" if bench else "ExternalOutput"
    x0o = nc.dram_tensor("x0o", [NS, CH], F32, kind=okind)
    x1o = nc.dram_tensor("x1o", [ES, CH], F32, kind=okind)
    dummy = (nc.dram_tensor("bench_out", [1, 4], F32, kind="ExternalOutput")
             if bench else None)

    with tile.TileContext(nc) as tc:
        with (
            tc.tile_pool(name="const", bufs=1) as const,
            tc.tile_pool(name="psum", bufs=8, space="PSUM") as psum,
            tc.tile_pool(name="dram", bufs=1, space="DRAM") as dram,
            tc.tile_pool(name="small", bufs=4) as small,
            tc.tile_pool(name="evac", bufs=4) as evac,
        ):
            # ---- constants / persistent tensors ----
            x0t_sb = const.tile([P, 2, NS], F32)
            nc.sync.dma_start(x0t_sb[:], x0t.rearrange("(k p) n -> p k n", p=P))
            w0_sb = const.tile([P, 2, CH], F32)
            nc.sync.dma_start(w0_sb[:], w0.rearrange("(k p) c -> p k c", p=P))
            w1_sb = const.tile([P, 2, CH], F32)
            nc.sync.dma_start(w1_sb[:], w1.rearrange("(k p) c -> p k c", p=P))
            w1bf_sb = const.tile([P, 2, CH], BF)
            nc.vector.tensor_copy(w1bf_sb[:], w1_sb[:])
            b01_sb = const.tile([P, CH], F32)
            nc.sync.dma_start(b01_sb[:], b01[:])
            b10_sb = const.tile([P, CH], F32)
            nc.sync.dma_start(b10_sb[:], b10[:])
            ident = const.tile([P, P], F32)
            make_identity(nc, ident[:])

            y_sb = const.tile([P, NB, CH], F32)       # x0 @ W0 (node shard)
            yp_sb = const.tile([P, NB, CH + 2], BF)   # [nc*y | nc | 1]
            x1t_sb = const.tile([P, 2, ES], BF)       # x1 shard, transposed
            zaug_sb = const.tile([P, EBF, CH + 1], BF)  # [ec*z | ec], all edges
            ecs_sb = const.tile([P, EBS], F32)        # edge_card per shard blk

            for _rep in range(reps):
                _emit_body(nc, tc, psum, dram, small, evac, locals())
            if bench:
                nc.sync.dma_start(dummy[:], ecs_sb[0:1, 0:4])

    nc.compile()
    return nc


# ReduceScatter and AllGather are issued in N_CHUNKS pieces overlapped with
# P2/P3.  Chunk c's AllGather output lands in zaug_sb blocks [CBLK*c,
# CBLK*(c+1)), ordered core-major inside; PERM maps a global edge block to
# its position in zaug_sb.
N_CHUNKS = 2
CBLK = EBF // N_CHUNKS          # 32 global blocks per chunk
CROWS = N_EDGES // N_CHUNKS     # 4096 edges per chunk
LROWS = CROWS // N_CORES        # 512 edges per (core, chunk)
LBLK = LROWS // P               # 4 local blocks per (core, chunk)

PERM = [CBLK * (B // CBLK) + LBLK * (B % CBLK // LBLK) + B % LBLK
        for B in range(EBF)]


def _emit_body(nc, tc, psum, dram, small, evac, env):
    b1r, b1t, x0o, x1o = env["b1r"], env["b1t"], env["x0o"], env["x1o"]
    x0t_sb, w0_sb, w1bf_sb = env["x0t_sb"], env["w0_sb"], env["w1bf_sb"]
    b01_sb, b10_sb, ident = env["b01_sb"], env["b10_sb"], env["ident"]
    y_sb, yp_sb, x1t_sb = env["y_sb"], env["yp_sb"], env["x1t_sb"]
    zaug_sb, ecs_sb = env["zaug_sb"], env["ecs_sb"]
    if True:
        if True:
            # ---- P0: y = x0 @ W0 ----
            for n in range(NB):
                ps = psum.tile([P, CH], F32, tag="ps", name=f"ps_y{n}")
                for k in range(2):
                    nc.tensor.matmul(ps[:], x0t_sb[:, k, n * P:(n + 1) * P],
                                     w0_sb[:, k, :], start=(k == 0),
                                     stop=(k == 1))
                nc.scalar.copy(y_sb[:, n, :], ps[:])

            # ---- P1: row sums -> node_card -> Yp ----
            # Tree of bf16 pairwise adds (values stay exact integers <= 16).
            # Alternate node blocks between DVE and GpSimd so the two engines
            # halve the wall time; an 8192-wide TensorReduce on DVE alone ran
            # at 1x mode and took 139us.
            with tc.tile_pool(name="rows", bufs=3) as rows, \
                 tc.tile_pool(name="tree", bufs=2) as tree:
                for n in range(NB):
                    eng = nc.vector if n % 2 == 0 else nc.gpsimd
                    rt = rows.tile([P, N_EDGES], BF, tag="rt", name=f"rt{n}")
                    nc.sync.dma_start(rt[:], b1r[n * P:(n + 1) * P, :])
                    t1 = tree.tile([P, 4096], BF, tag="t1", name=f"t1_{n}")
                    eng.tensor_tensor(t1[:], rt[:, :4096], rt[:, 4096:],
                                      ALU.add)
                    t2 = tree.tile([P, 2048], BF, tag="t2", name=f"t2_{n}")
                    eng.tensor_tensor(t2[:], t1[:, :2048], t1[:, 2048:],
                                      ALU.add)
                    t3 = tree.tile([P, 1024], BF, tag="t3", name=f"t3_{n}")
                    eng.tensor_tensor(t3[:], t2[:, :1024], t2[:, 1024:],
                                      ALU.add)
                    t4 = tree.tile([P, 512], BF, tag="t4", name=f"t4_{n}")
                    eng.tensor_tensor(t4[:], t3[:, :512], t3[:, 512:],
                                      ALU.add)
                    rs = small.tile([P, 1], F32, tag="rs", name=f"rs{n}")
                    nc.vector.reduce_sum(rs[:], t4[:], axis=AX.X)
                    ri = small.tile([P, 1], F32, tag="ri", name=f"ri{n}")
                    nc.vector.reciprocal(ri[:], rs[:])
                    ncard = small.tile([P, 1], F32, tag="ncard", name=f"nc{n}")
                    nc.scalar.sqrt(ncard[:], ri[:])
                    nc.scalar.activation(yp_sb[:, n, 0:CH], y_sb[:, n, :],
                                         mybir.ActivationFunctionType.Copy,
                                         scale=ncard[:])
                    nc.vector.tensor_copy(yp_sb[:, n, CH:CH + 1], ncard[:])
                    nc.vector.memset(yp_sb[:, n, CH + 1:CH + 2], 1.0)

            # ---- P2: U = B1^T @ Yp, accumulated over node blocks ----
            # One DRAM chunk tensor per future ReduceScatter piece so each
            # collective's dependencies cover only its two octs.
            u_chunks = [dram.tile([CROWS, CH + 2], F32, name=f"u_c{c}")
                        for c in range(N_CHUNKS)]
            urs, zloc, zagd = [], [], []

            def emit_tail(c):
                # Post-ReduceScatter work for chunk c: stats, x1 (+output),
                # x1^T, z = x1@W1, Zaug(local), then that chunk's AllGather.
                zl = dram.tile([LROWS, CH + 1], BF, name=f"zloc{c}")
                zloc.append(zl)
                for o in range(LBLK):
                    lb = c * LBLK + o
                    ut = evac.tile([P, CH + 2], F32, tag="ut", name=f"ut{lb}")
                    nc.sync.dma_start(ut[:], urs[c][o * P:(o + 1) * P, :])
                    d1i = small.tile([P, 1], F32, tag="d1i", name=f"d1i{lb}")
                    nc.vector.reciprocal(d1i[:], ut[:, CH:CH + 1])
                    ci = small.tile([P, 1], F32, tag="ci", name=f"ci{lb}")
                    nc.vector.reciprocal(ci[:], ut[:, CH + 1:CH + 2])
                    cs = small.tile([P, 1], F32, tag="cs", name=f"cs{lb}")
                    nc.scalar.sqrt(cs[:], ci[:])
                    nc.vector.tensor_tensor(ecs_sb[:, lb:lb + 1], ci[:],
                                            cs[:], ALU.mult)
                    x1 = evac.tile([P, CH], F32, tag="x1", name=f"x1_{lb}")
                    nc.vector.tensor_scalar_mul(x1[:], ut[:, 0:CH], d1i[:])
                    nc.vector.tensor_add(x1[:], x1[:], b01_sb[:])
                    x1r = evac.tile([P, CH], F32, tag="x1r", name=f"x1r{lb}")
                    nc.vector.tensor_scalar_max(x1r[:], x1[:], 0.0)
                    nc.sync.dma_start(x1o[lb * P:(lb + 1) * P, :], x1r[:])
                    for c2 in range(2):
                        tp = psum.tile([P, P], F32, tag="ps",
                                       name=f"tp{lb}_{c2}")
                        nc.tensor.transpose(tp[:], x1[:, c2 * P:(c2 + 1) * P],
                                            ident[:])
                        nc.vector.tensor_copy(
                            x1t_sb[:, c2, lb * P:(lb + 1) * P], tp[:])
                    zps = psum.tile([P, CH], F32, tag="ps", name=f"ps_z{lb}")
                    for c2 in range(2):
                        nc.tensor.matmul(
                            zps[:], x1t_sb[:, c2, lb * P:(lb + 1) * P],
                            w1bf_sb[:, c2, :], start=(c2 == 0),
                            stop=(c2 == 1))
                    zt = evac.tile([P, CH + 1], BF, tag="zt", name=f"zt{lb}")
                    nc.vector.tensor_scalar_mul(zt[:, 0:CH], zps[:],
                                                ecs_sb[:, lb:lb + 1])
                    nc.vector.tensor_copy(zt[:, CH:CH + 1],
                                          ecs_sb[:, lb:lb + 1])
                    nc.sync.dma_start(zl[o * P:(o + 1) * P, :], zt[:])
                zd = dram.tile([CROWS, CH + 1], BF, addr_space="Shared",
                               name=f"zaug_d{c}")
                zagd.append(zd)
                nc.gpsimd.collective_compute(
                    "AllGather", ALU.bypass, replica_groups=GROUPS,
                    ins=[zl[:].opt()], outs=[zd[:].opt()])
                nc.sync.dma_start(
                    zaug_sb[:, CBLK * c:CBLK * (c + 1), :],
                    zd.rearrange("(e p) ch -> p e ch", p=P))

            # 16 half-oct streaming tiles (1KB DMA lines, 4-deep prefetch).
            with tc.tile_pool(name="octp", bufs=4) as octp:
                for ho in range(16):
                    ot = octp.tile([P, NB, 512], BF, tag="oct",
                                   name=f"ho{ho}")
                    for n in range(NB):
                        nc.sync.dma_start(
                            ot[:, n, :],
                            b1r[n * P:(n + 1) * P, ho * 512:(ho + 1) * 512])
                    for e4 in range(4):
                        e = ho * 4 + e4
                        ups = psum.tile([P, CH + 2], F32, tag="ps",
                                        name=f"ps_u{e}")
                        for n in range(NB):
                            nc.tensor.matmul(
                                ups[:], ot[:, n, e4 * P:(e4 + 1) * P],
                                yp_sb[:, n, :],
                                start=(n == 0), stop=(n == NB - 1))
                        ue = evac.tile([P, CH + 2], F32, tag="ue",
                                       name=f"ue{e}")
                        nc.vector.tensor_copy(ue[:], ups[:])
                        c, cr = divmod(e, CBLK)
                        nc.sync.dma_start(
                            u_chunks[c][cr * P:(cr + 1) * P, :], ue[:])
                    # ReduceScatter a chunk once its half-octs are written;
                    # chunk 0's tail is emitted two half-octs later so its PE
                    # work doesn't stall the in-order PE stream while the
                    # collective is still in flight.
                    if ho == 7 or ho == 15:
                        c = ho // 8
                        urs_c = dram.tile([LROWS, CH + 2], F32,
                                          name=f"urs_c{c}")
                        urs.append(urs_c)
                        nc.gpsimd.collective_compute(
                            "ReduceScatter", ALU.add, replica_groups=GROUPS,
                            ins=[u_chunks[c][:].opt()],
                            outs=[urs_c[:].opt()])
                    if ho == 13:
                        emit_tail(0)
            emit_tail(1)

            # ---- P3: V = B1 @ Zaug, accumulated over edge blocks ----
            with tc.tile_pool(name="btp", bufs=8) as btp:
                for nh in range(2):
                    vps = []
                    for n8 in range(8):
                        vps.append(psum.tile([P, CH + 1], F32, tag="ps",
                                             name=f"ps_v{nh}_{n8}"))
                    for ebi in range(EBF):
                        bt = btp.tile([P, 1024], BF, tag="bt",
                                      name=f"bt{nh}_{ebi}")
                        nc.sync.dma_start(
                            bt[:], b1t[ebi * P:(ebi + 1) * P,
                                       nh * 1024:(nh + 1) * 1024])
                        for n8 in range(8):
                            nc.tensor.matmul(
                                vps[n8][:], bt[:, n8 * P:(n8 + 1) * P],
                                zaug_sb[:, PERM[ebi], :],
                                start=(ebi == 0), stop=(ebi == EBF - 1))
                    for n8 in range(8):
                        n = nh * 8 + n8
                        d0i = small.tile([P, 1], F32, tag="d0i",
                                         name=f"d0i{n}")
                        nc.vector.reciprocal(d0i[:], vps[n8][:, CH:CH + 1])
                        xo = evac.tile([P, CH], F32, tag="xo", name=f"xo{n}")
                        nc.vector.tensor_scalar_mul(xo[:], vps[n8][:, 0:CH],
                                                    d0i[:])
                        nc.vector.tensor_add(xo[:], xo[:], b10_sb[:])
                        nc.vector.tensor_scalar_max(xo[:], xo[:], 0.0)
                        nc.sync.dma_start(x0o[n * P:(n + 1) * P, :], xo[:])


_NC_CACHE = None


def _get_nc():
    global _NC_CACHE
    if _NC_CACHE is None:
        _NC_CACHE = build_bass()
    return _NC_CACHE


def kernel(x_0, incidence_1, W0, W1, bias_0_to_1, bias_1_to_0):
    x_0 = np.asarray(x_0, dtype=np.float32)
    b1 = np.asarray(incidence_1, dtype=np.float32)
    W0 = np.asarray(W0, dtype=np.float32)
    W1 = np.asarray(W1, dtype=np.float32)
    b01 = np.ascontiguousarray(np.broadcast_to(
        np.asarray(bias_0_to_1, dtype=np.float32).reshape(1, CH), (P, CH)))
    b10 = np.ascontiguousarray(np.broadcast_to(
        np.asarray(bias_1_to_0, dtype=np.float32).reshape(1, CH), (P, CH)))

    b1_bf = b1.astype(BF16)
    in_maps = []
    for i in range(N_CORES):
        rows = slice(i * NS, (i + 1) * NS)
        shard = b1_bf[rows]
        in_maps.append({
            "b1r": np.ascontiguousarray(shard),
            "b1t": np.ascontiguousarray(shard.T),
            "x0t": np.ascontiguousarray(x_0[rows].T),
            "w0": W0, "w1": W1, "b01": b01, "b10": b10,
        })

    import os
    nc = _get_nc()
    trace = os.environ.get("KERNEL_TRACE", "0") != "0"
    if trace:
        import profhook  # noqa: F401  (registers the axon NTFF hook)
    res = run_bass_kernel_spmd(nc, in_maps, core_ids=list(range(N_CORES)),
                               trace=trace)
    if trace:
        print(f"HW exec time: {res.exec_time_ns} ns")
        print(f"trace: {res.instructions_and_trace[1] if res.instructions_and_trace else None}")
    x0_out = np.concatenate([r["x0o"] for r in res.results], axis=0)
    # x1 rows on core j are [chunk0 | chunk1 | ...], each LROWS wide; global
    # edges for (j, chunk c) are [CROWS*c + LROWS*j, +LROWS).
    x1_out = np.empty((N_EDGES, CH), np.float32)
    for j in range(N_CORES):
        loc = res.results[j]["x1o"]
        for c in range(N_CHUNKS):
            g0 = CROWS * c + LROWS * j
            x1_out[g0:g0 + LROWS] = loc[LROWS * c:LROWS * (c + 1)]
    return x0_out, x1_out


# revision 24
# speedup vs baseline: 1.0685x; 1.0685x over previous
"""HNHN layer (hypergraph message passing) on 8 Trainium2 NeuronCores.

Math (reference, with B1 the binary node-edge incidence matrix):
    edge_card = colsum(B1)^-1.5          node_card = rowsum(B1)^-0.5
    d0_inv    = 1/(B1 @ edge_card)       d1_inv    = 1/(B1^T @ node_card)
    x1     = d1_inv * (B1^T @ (node_card * (x0 @ W0))) + b01
    x0_out = d0_inv * (B1  @ (edge_card * (x1 @ W1))) + b10
    returns (relu(x0_out), relu(x1))

Implementation: nodes are row-sharded 8 ways. Per core, three streaming
passes over the (bf16, exact for 0/1 values) incidence shard:
  P1: row sums (DVE reduce)  -> node_card
  P2: U = B1_shard^T @ [node_card*y | node_card | 1]   (PE, psum-accum)
      -> ReduceScatter(add) over cores gives each core its edge shard of
         [B1^T Yp | d1 | colsums]
  P3: V = B1_shard @ [edge_card*z | edge_card]          (PE, psum-accum)
      where z = x1_shard @ W1 is computed on the edge shard and
      AllGather'ed (bf16) to every core.
Stats ride along as extra matmul columns so no separate matvec passes are
needed. Only host-side prep: sharding, transposes, bf16 casts.
"""

import numpy as np
import ml_dtypes

import concourse.bass as bass
import concourse.mybir as mybir
import concourse.tile as tile
from concourse import bacc
from concourse.bass_utils import run_bass_kernel_spmd
from concourse.masks import make_identity

BF16 = ml_dtypes.bfloat16

N_NODES, N_EDGES = 16384, 8192
CH = 256
N_CORES = 8
NS = N_NODES // N_CORES      # 2048 nodes per core
ES = N_EDGES // N_CORES      # 1024 edges per core (edge shard)
P = 128
NB = NS // P                 # 16 node blocks / core
EBF = N_EDGES // P           # 64 edge blocks (full)
EBS = ES // P                # 8 edge blocks (shard)
F32 = mybir.dt.float32
BF = mybir.dt.bfloat16
AX = mybir.AxisListType
ALU = mybir.AluOpType
GROUPS = [list(range(N_CORES))]


def build_bass(reps=1, bench=False):
    nc = bacc.Bacc("TRN2", target_bir_lowering=False, debug=False,
                   num_devices=N_CORES)
    b1r = nc.dram_tensor("b1r", [NS, N_EDGES], BF, kind="ExternalInput")
    b1t = nc.dram_tensor("b1t", [N_EDGES, NS], BF, kind="ExternalInput")
    x0t = nc.dram_tensor("x0t", [CH, NS], F32, kind="ExternalInput")
    w0 = nc.dram_tensor("w0", [CH, CH], F32, kind="ExternalInput")
    w1 = nc.dram_tensor("w1", [CH, CH], F32, kind="ExternalInput")
    b01 = nc.dram_tensor("b01", [P, CH], F32, kind="ExternalInput")
    b10 = nc.dram_tensor("b10", [P, CH], F32, kind="ExternalInput")
    okind = "Internal" if bench else "ExternalOutput"
    x0o = nc.dram_tensor("x0o", [NS, CH], F32, kind=okind)
    x1o = nc.dram_tensor("x1o", [ES, CH], F32, kind=okind)
    dummy = (nc.dram_tensor("bench_out", [1, 4], F32, kind="ExternalOutput")
             if bench else None)

    with tile.TileContext(nc) as tc:
        with (
            tc.tile_pool(name="const", bufs=1) as const,
            tc.tile_pool(name="psum", bufs=8, space="PSUM") as psum,
            tc.tile_pool(name="dram", bufs=1, space="DRAM") as dram,
            tc.tile_pool(name="small", bufs=4) as small,
            tc.tile_pool(name="evac", bufs=4) as evac,
        ):
            # ---- constants / persistent tensors ----
            x0t_sb = const.tile([P, 2, NS], F32)
            nc.sync.dma_start(x0t_sb[:], x0t.rearrange("(k p) n -> p k n", p=P))
            w0_sb = const.tile([P, 2, CH], F32)
            nc.sync.dma_start(w0_sb[:], w0.rearrange("(k p) c -> p k c", p=P))
            w1_sb = const.tile([P, 2, CH], F32)
            nc.sync.dma_start(w1_sb[:], w1.rearrange("(k p) c -> p k c", p=P))
            w1bf_sb = const.tile([P, 2, CH], BF)
            nc.vector.tensor_copy(w1bf_sb[:], w1_sb[:])
            b01_sb = const.tile([P, CH], F32)
            nc.sync.dma_start(b01_sb[:], b01[:])
            b10_sb = const.tile([P, CH], F32)
            nc.sync.dma_start(b10_sb[:], b10[:])
            ident = const.tile([P, P], F32)
            make_identity(nc, ident[:])

            y_sb = const.tile([P, NB, CH], F32)       # x0 @ W0 (node shard)
            yp_sb = const.tile([P, NB, CH + 2], BF)   # [nc*y | nc | 1]
            x1t_sb = const.tile([P, 2, ES], BF)       # x1 shard, transposed
            zaug_sb = const.tile([P, EBF, CH + 1], BF)  # [ec*z | ec], all edges
            ecs_sb = const.tile([P, EBS], F32)        # edge_card per shard blk

            for _rep in range(reps):
                _emit_body(nc, tc, psum, dram, small, evac, locals())
            if bench:
                nc.sync.dma_start(dummy[:], ecs_sb[0:1, 0:4])

    nc.compile()
    return nc


# ReduceScatter and AllGather are issued in N_CHUNKS pieces overlapped with
# P2/P3.  Chunk c's AllGather output lands in zaug_sb blocks [CBLK*c,
# CBLK*(c+1)), ordered core-major inside; PERM maps a global edge block to
# its position in zaug_sb.
N_CHUNKS = 2
CBLK = EBF // N_CHUNKS          # 32 global blocks per chunk
CROWS = N_EDGES // N_CHUNKS     # 4096 edges per chunk
LROWS = CROWS // N_CORES        # 512 edges per (core, chunk)
LBLK = LROWS // P               # 4 local blocks per (core, chunk)

PERM = [CBLK * (B // CBLK) + LBLK * (B % CBLK // LBLK) + B % LBLK
        for B in range(EBF)]


def _emit_body(nc, tc, psum, dram, small, evac, env):
    b1r, b1t, x0o, x1o = env["b1r"], env["b1t"], env["x0o"], env["x1o"]
    x0t_sb, w0_sb, w1bf_sb = env["x0t_sb"], env["w0_sb"], env["w1bf_sb"]
    b01_sb, b10_sb, ident = env["b01_sb"], env["b10_sb"], env["ident"]
    y_sb, yp_sb, x1t_sb = env["y_sb"], env["yp_sb"], env["x1t_sb"]
    zaug_sb, ecs_sb = env["zaug_sb"], env["ecs_sb"]
    if True:
        if True:
            # ---- P0: y = x0 @ W0 ----
            for n in range(NB):
                ps = psum.tile([P, CH], F32, tag="ps", name=f"ps_y{n}")
                for k in range(2):
                    nc.tensor.matmul(ps[:], x0t_sb[:, k, n * P:(n + 1) * P],
                                     w0_sb[:, k, :], start=(k == 0),
                                     stop=(k == 1))
                nc.scalar.copy(y_sb[:, n, :], ps[:])

            # ---- P1: row sums -> node_card -> Yp ----
            # Tree of bf16 pairwise adds (values stay exact integers <= 16).
            # Alternate node blocks between DVE and GpSimd so the two engines
            # halve the wall time; an 8192-wide TensorReduce on DVE alone ran
            # at 1x mode and took 139us.
            with tc.tile_pool(name="rows", bufs=3) as rows, \
                 tc.tile_pool(name="tree", bufs=2) as tree:
                for n in range(NB):
                    eng = nc.vector
                    rt = rows.tile([P, N_EDGES], BF, tag="rt", name=f"rt{n}")
                    nc.sync.dma_start(rt[:], b1r[n * P:(n + 1) * P, :])
                    t1 = tree.tile([P, 4096], BF, tag="t1", name=f"t1_{n}")
                    eng.tensor_tensor(t1[:], rt[:, :4096], rt[:, 4096:],
                                      ALU.add)
                    t2 = tree.tile([P, 2048], BF, tag="t2", name=f"t2_{n}")
                    eng.tensor_tensor(t2[:], t1[:, :2048], t1[:, 2048:],
                                      ALU.add)
                    t3 = tree.tile([P, 1024], BF, tag="t3", name=f"t3_{n}")
                    eng.tensor_tensor(t3[:], t2[:, :1024], t2[:, 1024:],
                                      ALU.add)
                    t4 = tree.tile([P, 512], BF, tag="t4", name=f"t4_{n}")
                    eng.tensor_tensor(t4[:], t3[:, :512], t3[:, 512:],
                                      ALU.add)
                    rs = small.tile([P, 1], F32, tag="rs", name=f"rs{n}")
                    nc.vector.reduce_sum(rs[:], t4[:], axis=AX.X)
                    ri = small.tile([P, 1], F32, tag="ri", name=f"ri{n}")
                    nc.vector.reciprocal(ri[:], rs[:])
                    ncard = small.tile([P, 1], F32, tag="ncard", name=f"nc{n}")
                    nc.scalar.sqrt(ncard[:], ri[:])
                    nc.scalar.activation(yp_sb[:, n, 0:CH], y_sb[:, n, :],
                                         mybir.ActivationFunctionType.Copy,
                                         scale=ncard[:])
                    nc.vector.tensor_copy(yp_sb[:, n, CH:CH + 1], ncard[:])
                    nc.vector.memset(yp_sb[:, n, CH + 1:CH + 2], 1.0)

            # ---- P2: U = B1^T @ Yp, accumulated over node blocks ----
            # One DRAM chunk tensor per future ReduceScatter piece so each
            # collective's dependencies cover only its two octs.
            u_chunks = [dram.tile([CROWS, CH + 2], F32, name=f"u_c{c}")
                        for c in range(N_CHUNKS)]
            urs, zloc, zagd = [], [], []

            def emit_tail(c):
                # Post-ReduceScatter work for chunk c: stats, x1 (+output),
                # x1^T, z = x1@W1, Zaug(local), then that chunk's AllGather.
                zl = dram.tile([LROWS, CH + 1], BF, name=f"zloc{c}")
                zloc.append(zl)
                for o in range(LBLK):
                    lb = c * LBLK + o
                    ut = evac.tile([P, CH + 2], F32, tag="ut", name=f"ut{lb}")
                    nc.sync.dma_start(ut[:], urs[c][o * P:(o + 1) * P, :])
                    d1i = small.tile([P, 1], F32, tag="d1i", name=f"d1i{lb}")
                    nc.vector.reciprocal(d1i[:], ut[:, CH:CH + 1])
                    ci = small.tile([P, 1], F32, tag="ci", name=f"ci{lb}")
                    nc.vector.reciprocal(ci[:], ut[:, CH + 1:CH + 2])
                    cs = small.tile([P, 1], F32, tag="cs", name=f"cs{lb}")
                    nc.scalar.sqrt(cs[:], ci[:])
                    nc.vector.tensor_tensor(ecs_sb[:, lb:lb + 1], ci[:],
                                            cs[:], ALU.mult)
                    x1 = evac.tile([P, CH], F32, tag="x1", name=f"x1_{lb}")
                    nc.vector.tensor_scalar_mul(x1[:], ut[:, 0:CH], d1i[:])
                    nc.vector.tensor_add(x1[:], x1[:], b01_sb[:])
                    x1r = evac.tile([P, CH], F32, tag="x1r", name=f"x1r{lb}")
                    nc.vector.tensor_scalar_max(x1r[:], x1[:], 0.0)
                    nc.sync.dma_start(x1o[lb * P:(lb + 1) * P, :], x1r[:])
                    for c2 in range(2):
                        tp = psum.tile([P, P], F32, tag="ps",
                                       name=f"tp{lb}_{c2}")
                        nc.tensor.transpose(tp[:], x1[:, c2 * P:(c2 + 1) * P],
                                            ident[:])
                        nc.vector.tensor_copy(
                            x1t_sb[:, c2, lb * P:(lb + 1) * P], tp[:])
                    zps = psum.tile([P, CH], F32, tag="ps", name=f"ps_z{lb}")
                    for c2 in range(2):
                        nc.tensor.matmul(
                            zps[:], x1t_sb[:, c2, lb * P:(lb + 1) * P],
                            w1bf_sb[:, c2, :], start=(c2 == 0),
                            stop=(c2 == 1))
                    zt = evac.tile([P, CH + 1], BF, tag="zt", name=f"zt{lb}")
                    nc.vector.tensor_scalar_mul(zt[:, 0:CH], zps[:],
                                                ecs_sb[:, lb:lb + 1])
                    nc.vector.tensor_copy(zt[:, CH:CH + 1],
                                          ecs_sb[:, lb:lb + 1])
                    nc.sync.dma_start(zl[o * P:(o + 1) * P, :], zt[:])
                zd = dram.tile([CROWS, CH + 1], BF, addr_space="Shared",
                               name=f"zaug_d{c}")
                zagd.append(zd)
                nc.gpsimd.collective_compute(
                    "AllGather", ALU.bypass, replica_groups=GROUPS,
                    ins=[zl[:].opt()], outs=[zd[:].opt()])
                nc.sync.dma_start(
                    zaug_sb[:, CBLK * c:CBLK * (c + 1), :],
                    zd.rearrange("(e p) ch -> p e ch", p=P))

            # 16 half-oct streaming tiles (1KB DMA lines, 4-deep prefetch).
            with tc.tile_pool(name="octp", bufs=4) as octp:
                for ho in range(16):
                    ot = octp.tile([P, NB, 512], BF, tag="oct",
                                   name=f"ho{ho}")
                    for n in range(NB):
                        nc.sync.dma_start(
                            ot[:, n, :],
                            b1r[n * P:(n + 1) * P, ho * 512:(ho + 1) * 512])
                    for e4 in range(4):
                        e = ho * 4 + e4
                        ups = psum.tile([P, CH + 2], F32, tag="ps",
                                        name=f"ps_u{e}")
                        for n in range(NB):
                            nc.tensor.matmul(
                                ups[:], ot[:, n, e4 * P:(e4 + 1) * P],
                                yp_sb[:, n, :],
                                start=(n == 0), stop=(n == NB - 1))
                        ue = evac.tile([P, CH + 2], F32, tag="ue",
                                       name=f"ue{e}")
                        nc.vector.tensor_copy(ue[:], ups[:])
                        c, cr = divmod(e, CBLK)
                        nc.sync.dma_start(
                            u_chunks[c][cr * P:(cr + 1) * P, :], ue[:])
                    # ReduceScatter a chunk once its half-octs are written;
                    # chunk 0's tail is emitted two half-octs later so its PE
                    # work doesn't stall the in-order PE stream while the
                    # collective is still in flight.
                    if ho == 7 or ho == 15:
                        c = ho // 8
                        urs_c = dram.tile([LROWS, CH + 2], F32,
                                          name=f"urs_c{c}")
                        urs.append(urs_c)
                        nc.gpsimd.collective_compute(
                            "ReduceScatter", ALU.add, replica_groups=GROUPS,
                            ins=[u_chunks[c][:].opt()],
                            outs=[urs_c[:].opt()])
                    if ho == 13:
                        emit_tail(0)
            emit_tail(1)

            # ---- P3: V = B1 @ Zaug, accumulated over edge blocks ----
            with tc.tile_pool(name="btp", bufs=8) as btp:
                for nh in range(2):
                    vps = []
                    for n8 in range(8):
                        vps.append(psum.tile([P, CH + 1], F32, tag="ps",
                                             name=f"ps_v{nh}_{n8}"))
                    for ebi in range(EBF):
                        bt = btp.tile([P, 1024], BF, tag="bt",
                                      name=f"bt{nh}_{ebi}")
                        nc.sync.dma_start(
                            bt[:], b1t[ebi * P:(ebi + 1) * P,
                                       nh * 1024:(nh + 1) * 1024])
                        for n8 in range(8):
                            nc.tensor.matmul(
                                vps[n8][:], bt[:, n8 * P:(n8 + 1) * P],
                                zaug_sb[:, PERM[ebi], :],
                                start=(ebi == 0), stop=(ebi == EBF - 1))
                    for n8 in range(8):
                        n = nh * 8 + n8
                        d0i = small.tile([P, 1], F32, tag="d0i",
                                         name=f"d0i{n}")
                        nc.vector.reciprocal(d0i[:], vps[n8][:, CH:CH + 1])
                        xo = evac.tile([P, CH], F32, tag="xo", name=f"xo{n}")
                        nc.vector.tensor_scalar_mul(xo[:], vps[n8][:, 0:CH],
                                                    d0i[:])
                        nc.vector.tensor_add(xo[:], xo[:], b10_sb[:])
                        nc.vector.tensor_scalar_max(xo[:], xo[:], 0.0)
                        nc.sync.dma_start(x0o[n * P:(n + 1) * P, :], xo[:])


_NC_CACHE = None


def _get_nc():
    global _NC_CACHE
    if _NC_CACHE is None:
        _NC_CACHE = build_bass()
    return _NC_CACHE


def kernel(x_0, incidence_1, W0, W1, bias_0_to_1, bias_1_to_0):
    x_0 = np.asarray(x_0, dtype=np.float32)
    b1 = np.asarray(incidence_1, dtype=np.float32)
    W0 = np.asarray(W0, dtype=np.float32)
    W1 = np.asarray(W1, dtype=np.float32)
    b01 = np.ascontiguousarray(np.broadcast_to(
        np.asarray(bias_0_to_1, dtype=np.float32).reshape(1, CH), (P, CH)))
    b10 = np.ascontiguousarray(np.broadcast_to(
        np.asarray(bias_1_to_0, dtype=np.float32).reshape(1, CH), (P, CH)))

    b1_bf = b1.astype(BF16)
    in_maps = []
    for i in range(N_CORES):
        rows = slice(i * NS, (i + 1) * NS)
        shard = b1_bf[rows]
        in_maps.append({
            "b1r": np.ascontiguousarray(shard),
            "b1t": np.ascontiguousarray(shard.T),
            "x0t": np.ascontiguousarray(x_0[rows].T),
            "w0": W0, "w1": W1, "b01": b01, "b10": b10,
        })

    import os
    nc = _get_nc()
    trace = os.environ.get("KERNEL_TRACE", "0") != "0"
    if trace:
        import profhook  # noqa: F401  (registers the axon NTFF hook)
    res = run_bass_kernel_spmd(nc, in_maps, core_ids=list(range(N_CORES)),
                               trace=trace)
    if trace:
        print(f"HW exec time: {res.exec_time_ns} ns")
        print(f"trace: {res.instructions_and_trace[1] if res.instructions_and_trace else None}")
    x0_out = np.concatenate([r["x0o"] for r in res.results], axis=0)
    # x1 rows on core j are [chunk0 | chunk1 | ...], each LROWS wide; global
    # edges for (j, chunk c) are [CROWS*c + LROWS*j, +LROWS).
    x1_out = np.empty((N_EDGES, CH), np.float32)
    for j in range(N_CORES):
        loc = res.results[j]["x1o"]
        for c in range(N_CHUNKS):
            g0 = CROWS * c + LROWS * j
            x1_out[g0:g0 + LROWS] = loc[LROWS * c:LROWS * (c + 1)]
    return x0_out, x1_out


# revision 25
# speedup vs baseline: 1.0854x; 1.0158x over previous
"""HNHN layer (hypergraph message passing) on 8 Trainium2 NeuronCores.

Math (reference, with B1 the binary node-edge incidence matrix):
    edge_card = colsum(B1)^-1.5          node_card = rowsum(B1)^-0.5
    d0_inv    = 1/(B1 @ edge_card)       d1_inv    = 1/(B1^T @ node_card)
    x1     = d1_inv * (B1^T @ (node_card * (x0 @ W0))) + b01
    x0_out = d0_inv * (B1  @ (edge_card * (x1 @ W1))) + b10
    returns (relu(x0_out), relu(x1))

Implementation: nodes are row-sharded 8 ways. Per core, three streaming
passes over the (bf16, exact for 0/1 values) incidence shard:
  P1: row sums (DVE reduce)  -> node_card
  P2: U = B1_shard^T @ [node_card*y | node_card | 1]   (PE, psum-accum)
      -> ReduceScatter(add) over cores gives each core its edge shard of
         [B1^T Yp | d1 | colsums]
  P3: V = B1_shard @ [edge_card*z | edge_card]          (PE, psum-accum)
      where z = x1_shard @ W1 is computed on the edge shard and
      AllGather'ed (bf16) to every core.
Stats ride along as extra matmul columns so no separate matvec passes are
needed. Only host-side prep: sharding, transposes, bf16 casts.
"""

import numpy as np
import ml_dtypes

import concourse.bass as bass
import concourse.mybir as mybir
import concourse.tile as tile
from concourse import bacc
from concourse.bass_utils import run_bass_kernel_spmd
from concourse.masks import make_identity

BF16 = ml_dtypes.bfloat16

N_NODES, N_EDGES = 16384, 8192
CH = 256
N_CORES = 8
NS = N_NODES // N_CORES      # 2048 nodes per core
ES = N_EDGES // N_CORES      # 1024 edges per core (edge shard)
P = 128
NB = NS // P                 # 16 node blocks / core
EBF = N_EDGES // P           # 64 edge blocks (full)
EBS = ES // P                # 8 edge blocks (shard)
F32 = mybir.dt.float32
BF = mybir.dt.bfloat16
AX = mybir.AxisListType
ALU = mybir.AluOpType
GROUPS = [list(range(N_CORES))]


def build_bass(reps=1, bench=False):
    nc = bacc.Bacc("TRN2", target_bir_lowering=False, debug=False,
                   num_devices=N_CORES)
    b1r = nc.dram_tensor("b1r", [NS, N_EDGES], BF, kind="ExternalInput")
    b1t = nc.dram_tensor("b1t", [N_EDGES, NS], BF, kind="ExternalInput")
    x0t = nc.dram_tensor("x0t", [CH, NS], F32, kind="ExternalInput")
    w0 = nc.dram_tensor("w0", [CH, CH], F32, kind="ExternalInput")
    w1 = nc.dram_tensor("w1", [CH, CH], F32, kind="ExternalInput")
    b01 = nc.dram_tensor("b01", [P, CH], F32, kind="ExternalInput")
    b10 = nc.dram_tensor("b10", [P, CH], F32, kind="ExternalInput")
    okind = "Internal" if bench else "ExternalOutput"
    x0o = nc.dram_tensor("x0o", [NS, CH], F32, kind=okind)
    x1o = nc.dram_tensor("x1o", [ES, CH], F32, kind=okind)
    dummy = (nc.dram_tensor("bench_out", [1, 4], F32, kind="ExternalOutput")
             if bench else None)

    with tile.TileContext(nc) as tc:
        with (
            tc.tile_pool(name="const", bufs=1) as const,
            tc.tile_pool(name="psum", bufs=8, space="PSUM") as psum,
            tc.tile_pool(name="dram", bufs=1, space="DRAM") as dram,
            tc.tile_pool(name="small", bufs=4) as small,
            tc.tile_pool(name="evac", bufs=4) as evac,
        ):
            # ---- constants / persistent tensors ----
            x0t_sb = const.tile([P, 2, NS], F32)
            nc.sync.dma_start(x0t_sb[:], x0t.rearrange("(k p) n -> p k n", p=P))
            w0_sb = const.tile([P, 2, CH], F32)
            nc.sync.dma_start(w0_sb[:], w0.rearrange("(k p) c -> p k c", p=P))
            w1_sb = const.tile([P, 2, CH], F32)
            nc.sync.dma_start(w1_sb[:], w1.rearrange("(k p) c -> p k c", p=P))
            w1bf_sb = const.tile([P, 2, CH], BF)
            nc.vector.tensor_copy(w1bf_sb[:], w1_sb[:])
            b01_sb = const.tile([P, CH], F32)
            nc.sync.dma_start(b01_sb[:], b01[:])
            b10_sb = const.tile([P, CH], F32)
            nc.sync.dma_start(b10_sb[:], b10[:])
            ident = const.tile([P, P], F32)
            make_identity(nc, ident[:])

            y_sb = const.tile([P, NB, CH], F32)       # x0 @ W0 (node shard)
            yp_sb = const.tile([P, NB, CH + 2], BF)   # [nc*y | nc | 1]
            x1t_sb = const.tile([P, 2, ES], BF)       # x1 shard, transposed
            zaug_sb = const.tile([P, EBF, CH + 1], BF)  # [ec*z | ec], all edges
            ecs_sb = const.tile([P, EBS], F32)        # edge_card per shard blk

            for _rep in range(reps):
                _emit_body(nc, tc, psum, dram, small, evac, locals())
            if bench:
                nc.sync.dma_start(dummy[:], ecs_sb[0:1, 0:4])

    nc.compile()
    return nc


# ReduceScatter and AllGather are issued in N_CHUNKS pieces overlapped with
# P2/P3.  Chunk c's AllGather output lands in zaug_sb blocks [CBLK*c,
# CBLK*(c+1)), ordered core-major inside; PERM maps a global edge block to
# its position in zaug_sb.
N_CHUNKS = 2
CBLK = EBF // N_CHUNKS          # 32 global blocks per chunk
CROWS = N_EDGES // N_CHUNKS     # 4096 edges per chunk
LROWS = CROWS // N_CORES        # 512 edges per (core, chunk)
LBLK = LROWS // P               # 4 local blocks per (core, chunk)

PERM = [CBLK * (B // CBLK) + LBLK * (B % CBLK // LBLK) + B % LBLK
        for B in range(EBF)]


def _emit_body(nc, tc, psum, dram, small, evac, env):
    b1r, b1t, x0o, x1o = env["b1r"], env["b1t"], env["x0o"], env["x1o"]
    x0t_sb, w0_sb, w1bf_sb = env["x0t_sb"], env["w0_sb"], env["w1bf_sb"]
    b01_sb, b10_sb, ident = env["b01_sb"], env["b10_sb"], env["ident"]
    y_sb, yp_sb, x1t_sb = env["y_sb"], env["yp_sb"], env["x1t_sb"]
    zaug_sb, ecs_sb = env["zaug_sb"], env["ecs_sb"]
    if True:
        if True:
            # ---- P0: y = x0 @ W0 ----
            for n in range(NB):
                ps = psum.tile([P, CH], F32, tag="ps", name=f"ps_y{n}")
                for k in range(2):
                    nc.tensor.matmul(ps[:], x0t_sb[:, k, n * P:(n + 1) * P],
                                     w0_sb[:, k, :], start=(k == 0),
                                     stop=(k == 1))
                nc.scalar.copy(y_sb[:, n, :], ps[:])

            # ---- P1: row sums -> node_card -> Yp ----
            # Tree of bf16 pairwise adds (values stay exact integers <= 16).
            # Alternate node blocks between DVE and GpSimd so the two engines
            # halve the wall time; an 8192-wide TensorReduce on DVE alone ran
            # at 1x mode and took 139us.
            with tc.tile_pool(name="rows", bufs=3) as rows, \
                 tc.tile_pool(name="tree", bufs=2) as tree:
                for n in range(NB):
                    eng = nc.vector
                    rt = rows.tile([P, N_EDGES], BF, tag="rt", name=f"rt{n}")
                    nc.sync.dma_start(rt[:], b1r[n * P:(n + 1) * P, :])
                    t1 = tree.tile([P, 4096], BF, tag="t1", name=f"t1_{n}")
                    eng.tensor_tensor(t1[:], rt[:, :4096], rt[:, 4096:],
                                      ALU.add)
                    t2 = tree.tile([P, 2048], BF, tag="t2", name=f"t2_{n}")
                    eng.tensor_tensor(t2[:], t1[:, :2048], t1[:, 2048:],
                                      ALU.add)
                    t3 = tree.tile([P, 1024], BF, tag="t3", name=f"t3_{n}")
                    eng.tensor_tensor(t3[:], t2[:, :1024], t2[:, 1024:],
                                      ALU.add)
                    t4 = tree.tile([P, 512], BF, tag="t4", name=f"t4_{n}")
                    eng.tensor_tensor(t4[:], t3[:, :512], t3[:, 512:],
                                      ALU.add)
                    rs = small.tile([P, 1], F32, tag="rs", name=f"rs{n}")
                    nc.vector.reduce_sum(rs[:], t4[:], axis=AX.X)
                    ri = small.tile([P, 1], F32, tag="ri", name=f"ri{n}")
                    nc.vector.reciprocal(ri[:], rs[:])
                    ncard = small.tile([P, 1], F32, tag="ncard", name=f"nc{n}")
                    nc.scalar.sqrt(ncard[:], ri[:])
                    nc.scalar.activation(yp_sb[:, n, 0:CH], y_sb[:, n, :],
                                         mybir.ActivationFunctionType.Copy,
                                         scale=ncard[:])
                    nc.vector.tensor_copy(yp_sb[:, n, CH:CH + 1], ncard[:])
                    nc.vector.memset(yp_sb[:, n, CH + 1:CH + 2], 1.0)

            # ---- P2: U = B1^T @ Yp, accumulated over node blocks ----
            # One DRAM chunk tensor per future ReduceScatter piece so each
            # collective's dependencies cover only its two octs.
            u_chunks = [dram.tile([CROWS, CH + 2], F32, name=f"u_c{c}")
                        for c in range(N_CHUNKS)]
            urs, zloc, zagd = [], [], []

            def emit_tail(c):
                # Post-ReduceScatter work for chunk c: stats, x1 (+output),
                # x1^T, z = x1@W1, Zaug(local), then that chunk's AllGather.
                zl = dram.tile([LROWS, CH + 1], BF, name=f"zloc{c}")
                zloc.append(zl)
                for o in range(LBLK):
                    lb = c * LBLK + o
                    ut = evac.tile([P, CH + 2], F32, tag="ut", name=f"ut{lb}")
                    nc.sync.dma_start(ut[:], urs[c][o * P:(o + 1) * P, :])
                    d1i = small.tile([P, 1], F32, tag="d1i", name=f"d1i{lb}")
                    nc.vector.reciprocal(d1i[:], ut[:, CH:CH + 1])
                    ci = small.tile([P, 1], F32, tag="ci", name=f"ci{lb}")
                    nc.vector.reciprocal(ci[:], ut[:, CH + 1:CH + 2])
                    cs = small.tile([P, 1], F32, tag="cs", name=f"cs{lb}")
                    nc.scalar.sqrt(cs[:], ci[:])
                    nc.vector.tensor_tensor(ecs_sb[:, lb:lb + 1], ci[:],
                                            cs[:], ALU.mult)
                    x1 = evac.tile([P, CH], F32, tag="x1", name=f"x1_{lb}")
                    nc.vector.tensor_scalar_mul(x1[:], ut[:, 0:CH], d1i[:])
                    nc.vector.tensor_add(x1[:], x1[:], b01_sb[:])
                    x1r = evac.tile([P, CH], F32, tag="x1r", name=f"x1r{lb}")
                    nc.vector.tensor_scalar_max(x1r[:], x1[:], 0.0)
                    nc.sync.dma_start(x1o[lb * P:(lb + 1) * P, :], x1r[:])
                    for c2 in range(2):
                        tp = psum.tile([P, P], F32, tag="ps",
                                       name=f"tp{lb}_{c2}")
                        nc.tensor.transpose(tp[:], x1[:, c2 * P:(c2 + 1) * P],
                                            ident[:])
                        nc.vector.tensor_copy(
                            x1t_sb[:, c2, lb * P:(lb + 1) * P], tp[:])
                    zps = psum.tile([P, CH], F32, tag="ps", name=f"ps_z{lb}")
                    for c2 in range(2):
                        nc.tensor.matmul(
                            zps[:], x1t_sb[:, c2, lb * P:(lb + 1) * P],
                            w1bf_sb[:, c2, :], start=(c2 == 0),
                            stop=(c2 == 1))
                    zt = evac.tile([P, CH + 1], BF, tag="zt", name=f"zt{lb}")
                    nc.vector.tensor_scalar_mul(zt[:, 0:CH], zps[:],
                                                ecs_sb[:, lb:lb + 1])
                    nc.vector.tensor_copy(zt[:, CH:CH + 1],
                                          ecs_sb[:, lb:lb + 1])
                    nc.sync.dma_start(zl[o * P:(o + 1) * P, :], zt[:])
                zd = dram.tile([CROWS, CH + 1], BF, addr_space="Shared",
                               name=f"zaug_d{c}")
                zagd.append(zd)
                nc.gpsimd.collective_compute(
                    "AllGather", ALU.bypass, replica_groups=GROUPS,
                    ins=[zl[:].opt()], outs=[zd[:].opt()])
                nc.sync.dma_start(
                    zaug_sb[:, CBLK * c:CBLK * (c + 1), :],
                    zd.rearrange("(e p) ch -> p e ch", p=P))

            # 16 half-oct streaming tiles (1KB DMA lines, 4-deep prefetch).
            with tc.tile_pool(name="octp", bufs=4) as octp:
                for ho in range(16):
                    ot = octp.tile([P, NB, 512], BF, tag="oct",
                                   name=f"ho{ho}")
                    for n in range(NB):
                        nc.sync.dma_start(
                            ot[:, n, :],
                            b1r[n * P:(n + 1) * P, ho * 512:(ho + 1) * 512])
                    for e4 in range(4):
                        e = ho * 4 + e4
                        ups = psum.tile([P, CH + 2], F32, tag="ps",
                                        name=f"ps_u{e}")
                        for n in range(NB):
                            nc.tensor.matmul(
                                ups[:], ot[:, n, e4 * P:(e4 + 1) * P],
                                yp_sb[:, n, :],
                                start=(n == 0), stop=(n == NB - 1))
                        ue = evac.tile([P, CH + 2], F32, tag="ue",
                                       name=f"ue{e}")
                        nc.vector.tensor_copy(ue[:], ups[:])
                        c, cr = divmod(e, CBLK)
                        nc.sync.dma_start(
                            u_chunks[c][cr * P:(cr + 1) * P, :], ue[:])
                    # ReduceScatter a chunk once its half-octs are written;
                    # chunk 0's tail is emitted two half-octs later so its PE
                    # work doesn't stall the in-order PE stream while the
                    # collective is still in flight.
                    if ho == 7 or ho == 15:
                        c = ho // 8
                        urs_c = dram.tile([LROWS, CH + 2], F32,
                                          name=f"urs_c{c}")
                        urs.append(urs_c)
                        nc.gpsimd.collective_compute(
                            "ReduceScatter", ALU.add, replica_groups=GROUPS,
                            ins=[u_chunks[c][:].opt()],
                            outs=[urs_c[:].opt()])
                    if ho == 13:
                        emit_tail(0)

            # ---- P3 phase A: accumulate chunk-0 edge blocks into v_sb ----
            # Emitted BEFORE tail(1) so its psum tiles are allocated ahead of
            # tail(1)'s in the rotating pool — otherwise P3 couldn't claim a
            # psum slot until chunk 1's ReduceScatter finished.
            with tc.tile_pool(name="vsb", bufs=1) as vsbp, \
                 tc.tile_pool(name="btp", bufs=8) as btp:
                v_sb = vsbp.tile([P, NB, CH + 1], F32)
                for nh in range(2):
                    vps = []
                    for n8 in range(8):
                        vps.append(psum.tile([P, CH + 1], F32, tag="ps",
                                             name=f"ps_va{nh}_{n8}"))
                    for ebi in range(CBLK):
                        bt = btp.tile([P, 1024], BF, tag="bt",
                                      name=f"btA{nh}_{ebi}")
                        nc.sync.dma_start(
                            bt[:], b1t[ebi * P:(ebi + 1) * P,
                                       nh * 1024:(nh + 1) * 1024])
                        for n8 in range(8):
                            nc.tensor.matmul(
                                vps[n8][:], bt[:, n8 * P:(n8 + 1) * P],
                                zaug_sb[:, PERM[ebi], :],
                                start=(ebi == 0), stop=(ebi == CBLK - 1))
                    for n8 in range(8):
                        n = nh * 8 + n8
                        nc.vector.tensor_copy(v_sb[:, n, :], vps[n8][:])

                emit_tail(1)

                # ---- P3 phase B: chunk-1 edge blocks, merge, output ----
                for nh in range(2):
                    vps = []
                    for n8 in range(8):
                        vps.append(psum.tile([P, CH + 1], F32, tag="ps",
                                             name=f"ps_vb{nh}_{n8}"))
                    for ebi in range(CBLK, EBF):
                        bt = btp.tile([P, 1024], BF, tag="bt",
                                      name=f"btB{nh}_{ebi}")
                        nc.sync.dma_start(
                            bt[:], b1t[ebi * P:(ebi + 1) * P,
                                       nh * 1024:(nh + 1) * 1024])
                        for n8 in range(8):
                            nc.tensor.matmul(
                                vps[n8][:], bt[:, n8 * P:(n8 + 1) * P],
                                zaug_sb[:, PERM[ebi], :],
                                start=(ebi == CBLK), stop=(ebi == EBF - 1))
                    for n8 in range(8):
                        n = nh * 8 + n8
                        vt = evac.tile([P, CH + 1], F32, tag="vt",
                                       name=f"vt{n}")
                        nc.vector.tensor_tensor(vt[:], vps[n8][:],
                                                v_sb[:, n, :], ALU.add)
                        d0i = small.tile([P, 1], F32, tag="d0i",
                                         name=f"d0i{n}")
                        nc.vector.reciprocal(d0i[:], vt[:, CH:CH + 1])
                        xo = evac.tile([P, CH], F32, tag="xo", name=f"xo{n}")
                        nc.vector.tensor_scalar_mul(xo[:], vt[:, 0:CH],
                                                    d0i[:])
                        nc.vector.tensor_add(xo[:], xo[:], b10_sb[:])
                        nc.vector.tensor_scalar_max(xo[:], xo[:], 0.0)
                        nc.sync.dma_start(x0o[n * P:(n + 1) * P, :], xo[:])


_NC_CACHE = None


def _get_nc():
    global _NC_CACHE
    if _NC_CACHE is None:
        _NC_CACHE = build_bass()
    return _NC_CACHE


def kernel(x_0, incidence_1, W0, W1, bias_0_to_1, bias_1_to_0):
    x_0 = np.asarray(x_0, dtype=np.float32)
    b1 = np.asarray(incidence_1, dtype=np.float32)
    W0 = np.asarray(W0, dtype=np.float32)
    W1 = np.asarray(W1, dtype=np.float32)
    b01 = np.ascontiguousarray(np.broadcast_to(
        np.asarray(bias_0_to_1, dtype=np.float32).reshape(1, CH), (P, CH)))
    b10 = np.ascontiguousarray(np.broadcast_to(
        np.asarray(bias_1_to_0, dtype=np.float32).reshape(1, CH), (P, CH)))

    b1_bf = b1.astype(BF16)
    in_maps = []
    for i in range(N_CORES):
        rows = slice(i * NS, (i + 1) * NS)
        shard = b1_bf[rows]
        in_maps.append({
            "b1r": np.ascontiguousarray(shard),
            "b1t": np.ascontiguousarray(shard.T),
            "x0t": np.ascontiguousarray(x_0[rows].T),
            "w0": W0, "w1": W1, "b01": b01, "b10": b10,
        })

    import os
    nc = _get_nc()
    trace = os.environ.get("KERNEL_TRACE", "0") != "0"
    if trace:
        import profhook  # noqa: F401  (registers the axon NTFF hook)
    res = run_bass_kernel_spmd(nc, in_maps, core_ids=list(range(N_CORES)),
                               trace=trace)
    if trace:
        print(f"HW exec time: {res.exec_time_ns} ns")
        print(f"trace: {res.instructions_and_trace[1] if res.instructions_and_trace else None}")
    x0_out = np.concatenate([r["x0o"] for r in res.results], axis=0)
    # x1 rows on core j are [chunk0 | chunk1 | ...], each LROWS wide; global
    # edges for (j, chunk c) are [CROWS*c + LROWS*j, +LROWS).
    x1_out = np.empty((N_EDGES, CH), np.float32)
    for j in range(N_CORES):
        loc = res.results[j]["x1o"]
        for c in range(N_CHUNKS):
            g0 = CROWS * c + LROWS * j
            x1_out[g0:g0 + LROWS] = loc[LROWS * c:LROWS * (c + 1)]
    return x0_out, x1_out
